# revision 4
# baseline (speedup 1.0000x reference)
"""Trainium2 Bass kernel for nn_CombinedLossExp71 (combined distillation loss).

Sharding: data-parallel over B across 8 cores, codebook replicated.
Each core b handles batch row b (1500 tokens x 512 dims):
  - feature/triplet losses from natural-layout tiles (125 tokens x 512).
  - soft-token KL + VQ via codebook logits. PE computes 2*x.c in fp16
    (hosts pre-scales/transposes/packs the weights); the PSUM drain is a
    DVE tensor_tensor_reduce producing nl_neg = c2 - 2*x.c (= -logit)
    with an EXACT per-row min accumulated for free (the softmax stability
    point and the VQ min-distance, no subsampling).
  - e = exp(-nl_neg + min) on ACT (scale=-1, bias=min; args always <= ~0,
    no overflow possible), row-sum accumulated.
  - delta = l_t - l_s = nl_neg_s - nl_neg_t on Pool; dot = sum(e_t*delta)
    via a second DVE tensor_tensor_reduce (mult + add-accum), all fp16
    tensors with fp32 accumulators.
  Per-core outputs are 4 partial sums [feature, triplet, kl, vq]; the
  final masked-mean combination happens on host (scalar work only).

Self-contained: hardcodes shapes for B=8, T=1500, D=512, K=4096, STRIDE=320.
"""
import numpy as np

try:
    import concourse.bass as bass
except ImportError:  # environment fallback
    import sys

    sys.path.insert(0, "/opt/trn_rl_repo")
    import concourse.bass as bass

import concourse.tile as tile
from concourse import mybir
from concourse.bass_utils import run_bass_kernel_spmd

B, T, D, K = 8, 1500, 512, 4096
STRIDE = 320
P = 125          # tokens per tile (partition dim)
NT = T // P      # 12 tiles
KH = K // 2      # K half processed per PSUM round
NC = 8           # cores
F32 = mybir.dt.float32
F16 = mybir.dt.float16

Act = mybir.ActivationFunctionType
Alu = mybir.AluOpType
AxX = mybir.AxisListType.X


def _split_sync_waits(nc, max_waits=1):
    """This container's walrus supports only one embedded sync-wait per
    instruction; move excess waits onto inserted same-engine NoOps."""
    counter = [0]
    for f in nc.m.functions:
        for bb in f.blocks:
            insts = bb.instructions
            out = []
            changed = False
            for ins in insts:
                si = ins.sync_info
                waits = list(si.on_wait) if si is not None and si.on_wait else []
                if len(waits) > max_waits:
                    changed = True
                    extra, keep = waits[:-max_waits], waits[-max_waits:]
                    for j in range(0, len(extra), max_waits):
                        counter[0] += 1
                        nop = mybir.InstNoOp(
                            name=f"wsplit-{counter[0]}",
                            ins=[],
                            outs=[],
                            engine=ins.engine,
                        )
                        nop.sync_info = mybir.SyncInfo(
                            on_wait=extra[j : j + max_waits], on_update=[]
                        )
                        nc.register_instruction(nop, overwrite=True)
                        out.append(nop)
                    si.on_wait = keep
                out.append(ins)
            if changed:
                insts.clear()
                insts.extend(out)


def _build(dump=False):
    nc = bass.Bass()

    s_nat = nc.dram_tensor("s_nat", [T, D], F32, kind="ExternalInput")
    t_nat = nc.dram_tensor("t_nat", [T, D], F32, kind="ExternalInput")
    tp_nat = nc.dram_tensor("tp_nat", [T, D], F32, kind="ExternalInput")
    # packed stationary weights: [128, NT*2*4*125] fp16,
    # wpack[row, ((it*2+f)*4+d)*125+p] = 2*X_f[it*125+p, d*128+row]
    wpack = nc.dram_tensor("wpack", [128, NT * 1000], F16, kind="ExternalInput")
    # packed codebook: ctpack[row, d*K+k] = C[k, d*128+row], fp16
    ctpack = nc.dram_tensor("ctpack", [128, 4 * K], F16, kind="ExternalInput")
    c2b_in = nc.dram_tensor("c2b", [128, K], F32, kind="ExternalInput")
    mask_in = nc.dram_tensor("maskT", [P, NT], F32, kind="ExternalInput")
    out_d = nc.dram_tensor("partials", [1, 4], F32, kind="ExternalOutput")
    dbg_d = nc.dram_tensor("dbg", [P, 12 * NT], F32, kind="ExternalOutput") if dump else None

    with tile.TileContext(nc) as tc:
        with (
            tc.tile_pool(name="const", bufs=1) as const,
            tc.tile_pool(name="stats", bufs=1) as stats,
            tc.tile_pool(name="scr", bufs=2) as scrpool,
            tc.tile_pool(name="w", bufs=3) as wpool,
            tc.tile_pool(name="nat", bufs=2) as natpool,
            tc.tile_pool(name="diff", bufs=2) as diffpool,
            tc.tile_pool(name="sq", bufs=2) as sqpool,
            tc.tile_pool(name="nl", bufs=2) as nlpool,
            tc.tile_pool(name="et", bufs=2) as etpool,
            tc.tile_pool(name="es", bufs=2) as espool,
            tc.tile_pool(name="dl", bufs=2) as dlpool,
            tc.tile_pool(name="prod", bufs=2) as prodpool,
            tc.tile_pool(name="psum", bufs=2, space="PSUM") as psum,
        ):
            # ---- constants ----
            ct_sb = []
            for d in range(4):
                c = const.tile([128, K], F16, tag=f"ct{d}", name=f"ct{d}")
                nc.sync.dma_start(out=c[:], in_=ctpack[:, d * K : (d + 1) * K])
                ct_sb.append(c)
            c2b = const.tile([128, K], F32, tag="c2b", name="c2b")
            nc.sync.dma_start(out=c2b[:], in_=c2b_in[:])
            maskT = const.tile([P, NT], F32, tag="mask", name="maskT")
            nc.sync.dma_start(out=maskT[:], in_=mask_in[:])
            ones = const.tile([P, 1], F32, tag="ones", name="ones")
            nc.vector.memset(ones[:], 1.0)
            b_margin = const.tile([P, 1], F32, tag="b_margin", name="b_margin")
            nc.vector.memset(b_margin[:], 0.2)

            # ---- per-token stat arrays (col it per tile) ----
            def stat(name):
                return stats.tile([P, NT], F32, tag=name, name=name)

            nm_t_all = stat("nm_t")      # -max(l_t), exact
            nm_s_all = stat("nm_s")      # -max(l_s), exact
            se_t_all = stat("se_t")
            se_s_all = stat("se_s")
            dot_all = stat("dot")
            rsq_pos_all = stat("rsq_pos")
            rsq_neg_all = stat("rsq_neg")
            x2_all = stat("x2")

            for it in range(NT):
                tok = slice(it * P, (it + 1) * P)

                # packed stationary weights for this tile (both features)
                wt = wpool.tile([128, 1000], F16, tag="w", name=f"wt{it}")
                nc.sync.dma_start(
                    out=wt[:], in_=wpack[:, it * 1000 : (it + 1) * 1000]
                )

                # natural tiles + feature/triplet/x2 pieces
                s_t = natpool.tile([P, D], F32, tag="s_nat", name="s_t")
                t_t = natpool.tile([P, D], F32, tag="t_nat", name="t_t")
                tp_t = natpool.tile([P, D], F32, tag="tp_nat", name="tp_t")
                nc.sync.dma_start(out=s_t[:], in_=s_nat[tok, :])
                nc.sync.dma_start(out=t_t[:], in_=t_nat[tok, :])
                nc.sync.dma_start(out=tp_t[:], in_=tp_nat[tok, :])

                dpos = diffpool.tile([P, D], F32, tag="dpos", name="dpos")
                dneg = diffpool.tile([P, D], F32, tag="dneg", name="dneg")
                nc.gpsimd.tensor_sub(out=dpos[:], in0=s_t[:], in1=t_t[:])
                nc.gpsimd.tensor_sub(out=dneg[:], in0=s_t[:], in1=tp_t[:])
                sqs = sqpool.tile([P, D], F32, tag="sqscr", name="sqs")
                nc.scalar.activation(
                    out=sqs[:], in_=dpos[:], func=Act.Square,
                    accum_out=rsq_pos_all[:, it : it + 1],
                )
                sqs2 = sqpool.tile([P, D], F32, tag="sqscr", name="sqs2")
                nc.scalar.activation(
                    out=sqs2[:], in_=dneg[:], func=Act.Square,
                    accum_out=rsq_neg_all[:, it : it + 1],
                )
                sqs3 = sqpool.tile([P, D], F32, tag="sqscr", name="sqs3")
                nc.scalar.activation(
                    out=sqs3[:], in_=s_t[:], func=Act.Square,
                    accum_out=x2_all[:, it : it + 1],
                )

                # scr cols: 4-7 se halves (t0,t1,s0,s1); 8-9 dot halves
                scr = scrpool.tile([P, 10], F32, tag="scr", name="scr")
                nlneg = [
                    nlpool.tile([P, K], F16, tag="nl_t", name="nl_t"),
                    nlpool.tile([P, K], F16, tag="nl_s", name="nl_s"),
                ]
                et = etpool.tile([P, K], F16, tag="et", name="et")

                # 4 PSUM rounds: (t,h0) (t,h1) (s,h0) (s,h1)
                for f in range(2):
                    for half in range(2):
                        hs = slice(half * KH, (half + 1) * KH)
                        ps = psum.tile([P, KH], F32, tag="ps", name=f"ps_{f}{half}")
                        for d in range(4):
                            lhsT = wt[:, (f * 4 + d) * 125 : (f * 4 + d + 1) * 125]
                            for kc in range(4):
                                c0 = half * KH + kc * 512
                                nc.tensor.matmul(
                                    out=ps[:, kc * 512 : (kc + 1) * 512],
                                    lhsT=lhsT,
                                    rhs=ct_sb[d][:, c0 : c0 + 512],
                                    start=(d == 0),
                                    stop=(d == 3),
                                )
                        # nl_neg = c2 - 2xc (= -logit)
                        nc.vector.tensor_sub(
                            out=nlneg[f][:, hs], in0=c2b[:P, hs], in1=ps[:]
                        )
                    # exact full-row min of nl_neg (= -max logit): exp args
                    # are always <= ~0, overflow impossible by construction
                    nm_col = (nm_t_all if f == 0 else nm_s_all)[:, it : it + 1]
                    nc.vector.tensor_reduce(
                        out=nm_col, in_=nlneg[f][:], axis=AxX, op=Alu.min,
                    )
                    # e = exp(-nl_neg + nm) (+ row-sum halves into scr)
                    for half in range(2):
                        hs = slice(half * KH, (half + 1) * KH)
                        if f == 0:
                            dst = et[:, hs]
                        else:
                            dst = espool.tile([P, KH], F16, tag="es", name=f"es{half}")[:]
                        nc.scalar.activation(
                            out=dst, in_=nlneg[f][:, hs], func=Act.Exp,
                            scale=-1.0, bias=nm_col,
                            accum_out=scr[:, 4 + f * 2 + half : 5 + f * 2 + half],
                        )

                # delta + dot halves: dl = nl_neg_s - nl_neg_t = l_t - l_s;
                # dot = sum(e_t * dl) via scalar_tensor_tensor sum-accum
                for half in range(2):
                    hs = slice(half * KH, (half + 1) * KH)
                    dl = dlpool.tile([P, KH], F16, tag="dl", name=f"dl{half}")
                    nc.gpsimd.tensor_sub(
                        out=dl[:], in0=nlneg[1][:, hs], in1=nlneg[0][:, hs]
                    )
                    pr = prodpool.tile([P, KH], F16, tag="prod", name=f"prod{half}")
                    nc.vector.scalar_tensor_tensor(
                        out=pr[:], in0=et[:, hs], scalar=1.0, in1=dl[:],
                        op0=Alu.mult, op1=Alu.mult,
                        accum_out=scr[:, 8 + half : 9 + half],
                    )

                # combine half-sums
                nc.gpsimd.tensor_add(
                    out=se_t_all[:, it : it + 1], in0=scr[:, 4:5], in1=scr[:, 5:6]
                )
                nc.gpsimd.tensor_add(
                    out=se_s_all[:, it : it + 1], in0=scr[:, 6:7], in1=scr[:, 7:8]
                )
                nc.gpsimd.tensor_add(
                    out=dot_all[:, it : it + 1], in0=scr[:, 8:9], in1=scr[:, 9:10]
                )

            # ---- final combine over (P, NT) ----
            def ftile(name):
                return stats.tile([P, NT], F32, tag=name, name=name)

            recip_t = ftile("recip_t")
            nc.vector.reciprocal(out=recip_t[:], in_=se_t_all[:])

            # full-range ln via exponent/mantissa split (ACT Ln table only
            # covers a narrow input range): ln(x) = Ln(m) + (e-127)*ln2
            LN2 = 0.6931471805599453
            I32 = mybir.dt.int32

            def full_ln(dst, src, pfx):
                ei = stats.tile([P, NT], I32, tag=pfx + "_ei", name=pfx + "_ei")
                nc.vector.tensor_scalar(
                    out=ei[:], in0=src[:].bitcast(I32), scalar1=23, scalar2=None,
                    op0=Alu.logical_shift_right,
                )
                ef = stats.tile([P, NT], F32, tag=pfx + "_ef", name=pfx + "_ef")
                nc.vector.tensor_copy(out=ef[:], in_=ei[:])
                mi = stats.tile([P, NT], I32, tag=pfx + "_mi", name=pfx + "_mi")
                nc.vector.tensor_scalar(
                    out=mi[:], in0=src[:].bitcast(I32),
                    scalar1=0x007FFFFF, scalar2=0x3F800000,
                    op0=Alu.bitwise_and, op1=Alu.bitwise_or,
                )
                nc.scalar.activation(out=dst[:], in_=mi[:].bitcast(F32), func=Act.Ln)
                ef2 = stats.tile([P, NT], F32, tag=pfx + "_ef2", name=pfx + "_ef2")
                nc.vector.tensor_scalar(
                    out=ef2[:], in0=ef[:], scalar1=LN2, scalar2=127.0 * LN2,
                    op0=Alu.mult, op1=Alu.subtract,
                )
                nc.vector.tensor_add(out=dst[:], in0=dst[:], in1=ef2[:])

            ln_ses = ftile("ln_ses")
            full_ln(ln_ses, se_s_all, "ls")
            ln_set = ftile("ln_set")
            full_ln(ln_set, se_t_all, "lt")
            kl = ftile("kl")
            nc.vector.tensor_mul(out=kl[:], in0=dot_all[:], in1=recip_t[:])
            nc.vector.tensor_add(out=kl[:], in0=kl[:], in1=nm_t_all[:])
            nc.vector.tensor_sub(out=kl[:], in0=kl[:], in1=nm_s_all[:])
            nc.vector.tensor_add(out=kl[:], in0=kl[:], in1=ln_ses[:])
            nc.vector.tensor_sub(out=kl[:], in0=kl[:], in1=ln_set[:])
            nc.vector.tensor_mul(out=kl[:], in0=kl[:], in1=maskT[:])

            packed = stats.tile([P, 4], F32, tag="packed", name="packed")
            fm = ftile("fm")
            nc.vector.tensor_mul(out=fm[:], in0=rsq_pos_all[:], in1=maskT[:])
            nc.vector.reduce_sum(out=packed[:, 0:1], in_=fm[:], axis=AxX)

            posd = ftile("posd")
            nc.scalar.activation(out=posd[:], in_=rsq_pos_all[:], func=Act.Sqrt)
            negd = ftile("negd")
            nc.scalar.activation(out=negd[:], in_=rsq_neg_all[:], func=Act.Sqrt)
            trip = ftile("trip")
            nc.vector.tensor_sub(out=trip[:], in0=posd[:], in1=negd[:])
            nc.scalar.activation(out=trip[:], in_=trip[:], func=Act.Relu, bias=b_margin[:])
            nc.vector.tensor_mul(out=trip[:], in0=trip[:], in1=maskT[:])
            nc.vector.reduce_sum(out=packed[:, 1:2], in_=trip[:], axis=AxX)

            nc.vector.reduce_sum(out=packed[:, 2:3], in_=kl[:], axis=AxX)

            vq = ftile("vq")
            nc.vector.tensor_add(out=vq[:], in0=x2_all[:], in1=nm_s_all[:])
            nc.vector.reduce_sum(out=packed[:, 3:4], in_=vq[:], axis=AxX)

            # partition reduce via fp32 matmul with ones
            pfin = psum.tile([1, 4], F32, tag="ps", name="pfin")
            nc.tensor.matmul(out=pfin[:], lhsT=ones[:], rhs=packed[:], start=True, stop=True)
            out_sb = stats.tile([1, 4], F32, tag="out_sb", name="out_sb")
            nc.scalar.copy(out=out_sb[:], in_=pfin[:])
            nc.sync.dma_start(out=out_d[:], in_=out_sb[:])

            if dump:
                arrs = [nm_t_all, nm_s_all, se_t_all, se_s_all, dot_all,
                        rsq_pos_all, rsq_neg_all, x2_all,
                        recip_t, ln_ses, ln_set, kl]
                for ai, arr in enumerate(arrs):
                    nc.sync.dma_start(
                        out=dbg_d[:, ai * NT : (ai + 1) * NT], in_=arr[:]
                    )

    _split_sync_waits(nc)
    return nc


_NC_CACHE = {}


def _get_nc(dump=False):
    key = "dump" if dump else "nc"
    if key not in _NC_CACHE:
        _NC_CACHE[key] = _build(dump=dump)
    return _NC_CACHE[key]


def _pack_w(X2):
    """(T, D) fp32 pre-scaled -> [128, NT*4*125] fp16 chunks per (it, d)."""
    # arr[it, p, d, row] -> [row? ] ; want w[row, it, d, p]
    a = X2.reshape(NT, P, 4, 128).transpose(3, 0, 2, 1)  # (128, NT, 4, P)
    return np.ascontiguousarray(a.astype(np.float16))


def kernel(student_features, teacher_features, teacher_codes, codebook, lengths,
           _debug=False, _trace=False, _dump=False):
    S = np.ascontiguousarray(np.asarray(student_features, dtype=np.float32))
    Tt = np.ascontiguousarray(np.asarray(teacher_features, dtype=np.float32))
    C = np.ascontiguousarray(np.asarray(codebook, dtype=np.float32))
    lengths = np.asarray(lengths)

    valid = np.minimum(lengths.astype(np.int64) // STRIDE, T)
    mask = (np.arange(T)[None, :] < valid[:, None]).astype(np.float32)  # (B,T)
    msum = float(mask.sum(dtype=np.float64))

    # packed codebook [128, 4*K] fp16 and c2 row
    ctpack = np.ascontiguousarray(
        C.reshape(K, 4, 128).transpose(2, 1, 0).reshape(128, 4 * K)
    ).astype(np.float16)
    c2 = (C.astype(np.float64) ** 2).sum(1).astype(np.float32)
    c2b = np.ascontiguousarray(np.broadcast_to(c2[None, :], (128, K)))

    in_maps = []
    for b in range(B):
        wt = _pack_w(2.0 * Tt[b])   # f=0: teacher
        ws = _pack_w(2.0 * S[b])    # f=1: student
        # interleave per tile: [row, it, f, d, p] -> [row, NT*1000]
        wpack = np.stack([wt, ws], axis=2)  # (128, NT, 2, 4, P)
        wpack = np.ascontiguousarray(wpack.reshape(128, NT * 1000))
        in_maps.append(
            {
                "s_nat": np.ascontiguousarray(S[b]),
                "t_nat": np.ascontiguousarray(Tt[b]),
                "tp_nat": np.ascontiguousarray(Tt[(b - 1) % B]),
                "wpack": wpack,
                "ctpack": ctpack,
                "c2b": c2b,
                "maskT": np.ascontiguousarray(mask[b].reshape(NT, P).T),
            }
        )

    nc = _get_nc(dump=_dump)
    res = run_bass_kernel_spmd(nc, in_maps, core_ids=list(range(NC)), trace=_trace)
    parts = np.stack([res.results[b]["partials"][0] for b in range(B)])  # (B,4)
    if _dump:
        dbg = np.stack([res.results[b]["dbg"] for b in range(B)])
        return parts, dbg

    F_sum, TR_sum, KL_sum, Q_sum = parts.astype(np.float64).sum(0)
    total = (
        F_sum / D / msum
        + TR_sum / msum
        + KL_sum / msum
        + 0.2 * Q_sum / (B * T * D)
    )
    out = np.array(total, dtype=np.float32)
    if _debug and _trace:
        return out, parts, res.exec_time_ns
    if _debug:
        return out, parts
    return out


# revision 6
# speedup vs baseline: 1.1522x; 1.1522x over previous
"""Trainium2 Bass kernel for nn_CombinedLossExp71 (combined distillation loss).

Sharding: data-parallel over B across 8 cores, codebook replicated.
Each core b handles batch row b (1500 tokens x 512 dims):
  - feature/triplet losses from natural-layout tiles (125 tokens x 512).
  - soft-token KL + VQ via codebook logits. PE computes 2*x.c in fp16
    (hosts pre-scales/transposes/packs the weights); the PSUM drain is a
    DVE tensor_tensor_reduce producing nl_neg = c2 - 2*x.c (= -logit)
    with an EXACT per-row min accumulated for free (the softmax stability
    point and the VQ min-distance, no subsampling).
  - e = exp(-nl_neg + min) on ACT (scale=-1, bias=min; args always <= ~0,
    no overflow possible), row-sum accumulated.
  - delta = l_t - l_s = nl_neg_s - nl_neg_t on Pool; dot = sum(e_t*delta)
    via a second DVE tensor_tensor_reduce (mult + add-accum), all fp16
    tensors with fp32 accumulators.
  Per-core outputs are 4 partial sums [feature, triplet, kl, vq]; the
  final masked-mean combination happens on host (scalar work only).

Self-contained: hardcodes shapes for B=8, T=1500, D=512, K=4096, STRIDE=320.
"""
import numpy as np

try:
    import concourse.bass as bass
except ImportError:  # environment fallback
    import sys

    sys.path.insert(0, "/opt/trn_rl_repo")
    import concourse.bass as bass

import concourse.tile as tile
from concourse import mybir
from concourse.bass_utils import run_bass_kernel_spmd

B, T, D, K = 8, 1500, 512, 4096
STRIDE = 320
P = 125          # tokens per tile (partition dim)
NT = T // P      # 12 tiles
KH = K // 2      # K half processed per PSUM round
NC = 8           # cores
F32 = mybir.dt.float32
F16 = mybir.dt.float16

Act = mybir.ActivationFunctionType
Alu = mybir.AluOpType
AxX = mybir.AxisListType.X


def _split_sync_waits(nc, max_waits=1):
    """This container's walrus supports only one embedded sync-wait per
    instruction; move excess waits onto inserted same-engine NoOps."""
    counter = [0]
    for f in nc.m.functions:
        for bb in f.blocks:
            insts = bb.instructions
            out = []
            changed = False
            for ins in insts:
                si = ins.sync_info
                waits = list(si.on_wait) if si is not None and si.on_wait else []
                if len(waits) > max_waits:
                    changed = True
                    extra, keep = waits[:-max_waits], waits[-max_waits:]
                    for j in range(0, len(extra), max_waits):
                        counter[0] += 1
                        nop = mybir.InstNoOp(
                            name=f"wsplit-{counter[0]}",
                            ins=[],
                            outs=[],
                            engine=ins.engine,
                        )
                        nop.sync_info = mybir.SyncInfo(
                            on_wait=extra[j : j + max_waits], on_update=[]
                        )
                        nc.register_instruction(nop, overwrite=True)
                        out.append(nop)
                    si.on_wait = keep
                out.append(ins)
            if changed:
                insts.clear()
                insts.extend(out)


def _build(dump=False):
    nc = bass.Bass()

    s_nat = nc.dram_tensor("s_nat", [T, D], F32, kind="ExternalInput")
    t_nat = nc.dram_tensor("t_nat", [T, D], F32, kind="ExternalInput")
    tp_nat = nc.dram_tensor("tp_nat", [T, D], F32, kind="ExternalInput")
    # packed stationary weights: [128, NT*2*4*125] fp16,
    # wpack[row, ((it*2+f)*4+d)*125+p] = 2*X_f[it*125+p, d*128+row]
    wpack = nc.dram_tensor("wpack", [128, NT * 1000], F16, kind="ExternalInput")
    # packed codebook: ctpack[row, d*K+k] = C[k, d*128+row], fp16
    ctpack = nc.dram_tensor("ctpack", [128, 4 * K], F16, kind="ExternalInput")
    c2b_in = nc.dram_tensor("c2b", [128, K], F32, kind="ExternalInput")
    mask_in = nc.dram_tensor("maskT", [P, NT], F32, kind="ExternalInput")
    out_d = nc.dram_tensor("partials", [1, 4], F32, kind="ExternalOutput")
    dbg_d = nc.dram_tensor("dbg", [P, 12 * NT], F32, kind="ExternalOutput") if dump else None

    with tile.TileContext(nc) as tc:
        with (
            tc.tile_pool(name="const", bufs=1) as const,
            tc.tile_pool(name="stats", bufs=1) as stats,
            tc.tile_pool(name="scr", bufs=2) as scrpool,
            tc.tile_pool(name="w", bufs=3) as wpool,
            tc.tile_pool(name="nat", bufs=2) as natpool,
            tc.tile_pool(name="diff", bufs=2) as diffpool,
            tc.tile_pool(name="sq", bufs=2) as sqpool,
            tc.tile_pool(name="nl", bufs=3) as nlpool,
            tc.tile_pool(name="et", bufs=2) as etpool,
            tc.tile_pool(name="es", bufs=2) as espool,
            tc.tile_pool(name="dl", bufs=2) as dlpool,
            tc.tile_pool(name="prod", bufs=2) as prodpool,
            tc.tile_pool(name="psum", bufs=2, space="PSUM") as psum,
        ):
            # ---- constants ----
            ct_sb = []
            for d in range(4):
                c = const.tile([128, K], F16, tag=f"ct{d}", name=f"ct{d}")
                nc.sync.dma_start(out=c[:], in_=ctpack[:, d * K : (d + 1) * K])
                ct_sb.append(c)
            c2b = const.tile([128, K], F32, tag="c2b", name="c2b")
            nc.sync.dma_start(out=c2b[:], in_=c2b_in[:])
            maskT = const.tile([P, NT], F32, tag="mask", name="maskT")
            nc.sync.dma_start(out=maskT[:], in_=mask_in[:])
            ones = const.tile([P, 1], F32, tag="ones", name="ones")
            nc.vector.memset(ones[:], 1.0)
            b_margin = const.tile([P, 1], F32, tag="b_margin", name="b_margin")
            nc.vector.memset(b_margin[:], 0.2)

            # ---- per-token stat arrays (col it per tile) ----
            def stat(name):
                return stats.tile([P, NT], F32, tag=name, name=name)

            nm_t_all = stat("nm_t")      # -max(l_t), exact
            nm_s_all = stat("nm_s")      # -max(l_s), exact
            se_t_all = stat("se_t")
            se_s_all = stat("se_s")
            dot_all = stat("dot")
            rsq_pos_all = stat("rsq_pos")
            rsq_neg_all = stat("rsq_neg")
            x2_all = stat("x2")

            for it in range(NT):
                tok = slice(it * P, (it + 1) * P)

                # packed stationary weights for this tile (both features)
                wt = wpool.tile([128, 1000], F16, tag="w", name=f"wt{it}")
                nc.sync.dma_start(
                    out=wt[:], in_=wpack[:, it * 1000 : (it + 1) * 1000]
                )

                # natural tiles + feature/triplet/x2 pieces
                s_t = natpool.tile([P, D], F32, tag="s_nat", name="s_t")
                t_t = natpool.tile([P, D], F32, tag="t_nat", name="t_t")
                tp_t = natpool.tile([P, D], F32, tag="tp_nat", name="tp_t")
                nc.sync.dma_start(out=s_t[:], in_=s_nat[tok, :])
                nc.sync.dma_start(out=t_t[:], in_=t_nat[tok, :])
                nc.sync.dma_start(out=tp_t[:], in_=tp_nat[tok, :])

                dpos = diffpool.tile([P, D], F32, tag="dpos", name="dpos")
                dneg = diffpool.tile([P, D], F32, tag="dneg", name="dneg")
                nc.gpsimd.tensor_sub(out=dpos[:], in0=s_t[:], in1=t_t[:])
                nc.gpsimd.tensor_sub(out=dneg[:], in0=s_t[:], in1=tp_t[:])
                sqs = sqpool.tile([P, D], F32, tag="sqscr", name="sqs")
                nc.scalar.activation(
                    out=sqs[:], in_=dpos[:], func=Act.Square,
                    accum_out=rsq_pos_all[:, it : it + 1],
                )
                sqs2 = sqpool.tile([P, D], F32, tag="sqscr", name="sqs2")
                nc.scalar.activation(
                    out=sqs2[:], in_=dneg[:], func=Act.Square,
                    accum_out=rsq_neg_all[:, it : it + 1],
                )
                sqs3 = sqpool.tile([P, D], F32, tag="sqscr", name="sqs3")
                nc.scalar.activation(
                    out=sqs3[:], in_=s_t[:], func=Act.Square,
                    accum_out=x2_all[:, it : it + 1],
                )

                # scr cols: 0-3 min halves; 4-7 se halves (t0,t1,s0,s1);
                # 8-9 dot halves
                scr = scrpool.tile([P, 10], F32, tag="scr", name="scr")
                nlneg = [
                    nlpool.tile([P, K], F16, tag="nl_t", name="nl_t"),
                    nlpool.tile([P, K], F16, tag="nl_s", name="nl_s"),
                ]
                # dedicated per-half e_t tiles (an offset slice of one big
                # tile makes the downstream STT run ~2.7x slower)
                et = [
                    etpool.tile([P, KH], F16, tag="et0", name="et0"),
                    etpool.tile([P, KH], F16, tag="et1", name="et1"),
                ]

                # 4 PSUM rounds: (t,h0) (t,h1) (s,h0) (s,h1)
                for f in range(2):
                    nm_col = (nm_t_all if f == 0 else nm_s_all)[:, it : it + 1]
                    for half in range(2):
                        hs = slice(half * KH, (half + 1) * KH)
                        ps = psum.tile([P, KH], F32, tag="ps", name=f"ps_{f}{half}")
                        for d in range(4):
                            lhsT = wt[:, (f * 4 + d) * 125 : (f * 4 + d + 1) * 125]
                            for kc in range(4):
                                c0 = half * KH + kc * 512
                                nc.tensor.matmul(
                                    out=ps[:, kc * 512 : (kc + 1) * 512],
                                    lhsT=lhsT,
                                    rhs=ct_sb[d][:, c0 : c0 + 512],
                                    start=(d == 0),
                                    stop=(d == 3),
                                )
                        # nl_neg = c2 - 2xc (= -logit)
                        nc.vector.tensor_sub(
                            out=nlneg[f][:, hs], in0=c2b[:P, hs], in1=ps[:]
                        )
                        # exact per-half row min (overlaps the other half's
                        # drain; combined below)
                        nc.vector.tensor_reduce(
                            out=scr[:, f * 2 + half : f * 2 + half + 1],
                            in_=nlneg[f][:, hs], axis=AxX, op=Alu.min,
                        )
                    # nm = min over both halves (= -max logit, exact): exp
                    # args are always <= ~0, overflow impossible
                    nc.vector.tensor_tensor(
                        out=nm_col, in0=scr[:, f * 2 : f * 2 + 1],
                        in1=scr[:, f * 2 + 1 : f * 2 + 2], op=Alu.min,
                    )
                    # e = exp(-nl_neg + nm) (+ row-sum halves into scr)
                    for half in range(2):
                        hs = slice(half * KH, (half + 1) * KH)
                        if f == 0:
                            dst = et[half][:]
                        else:
                            dst = espool.tile([P, KH], F16, tag="es", name=f"es{half}")[:]
                        nc.scalar.activation(
                            out=dst, in_=nlneg[f][:, hs], func=Act.Exp,
                            scale=-1.0, bias=nm_col,
                            accum_out=scr[:, 4 + f * 2 + half : 5 + f * 2 + half],
                        )

                # delta + dot halves: dl = nl_neg_s - nl_neg_t = l_t - l_s;
                # dot = sum(e_t * dl) via scalar_tensor_tensor sum-accum
                for half in range(2):
                    hs = slice(half * KH, (half + 1) * KH)
                    dl = dlpool.tile([P, KH], F16, tag="dl", name=f"dl{half}")
                    nc.gpsimd.tensor_sub(
                        out=dl[:], in0=nlneg[1][:, hs], in1=nlneg[0][:, hs]
                    )
                    pr = prodpool.tile([P, KH], F16, tag="prod", name=f"prod{half}")
                    nc.vector.scalar_tensor_tensor(
                        out=pr[:], in0=et[half][:], scalar=1.0, in1=dl[:],
                        op0=Alu.mult, op1=Alu.mult,
                        accum_out=scr[:, 8 + half : 9 + half],
                    )

                # combine half-sums
                nc.gpsimd.tensor_add(
                    out=se_t_all[:, it : it + 1], in0=scr[:, 4:5], in1=scr[:, 5:6]
                )
                nc.gpsimd.tensor_add(
                    out=se_s_all[:, it : it + 1], in0=scr[:, 6:7], in1=scr[:, 7:8]
                )
                nc.gpsimd.tensor_add(
                    out=dot_all[:, it : it + 1], in0=scr[:, 8:9], in1=scr[:, 9:10]
                )

            # ---- final combine over (P, NT) ----
            def ftile(name):
                return stats.tile([P, NT], F32, tag=name, name=name)

            recip_t = ftile("recip_t")
            nc.vector.reciprocal(out=recip_t[:], in_=se_t_all[:])

            # full-range ln via exponent/mantissa split (ACT Ln table only
            # covers a narrow input range): ln(x) = Ln(m) + (e-127)*ln2
            LN2 = 0.6931471805599453
            I32 = mybir.dt.int32

            def full_ln(dst, src, pfx):
                ei = stats.tile([P, NT], I32, tag=pfx + "_ei", name=pfx + "_ei")
                nc.vector.tensor_scalar(
                    out=ei[:], in0=src[:].bitcast(I32), scalar1=23, scalar2=None,
                    op0=Alu.logical_shift_right,
                )
                ef = stats.tile([P, NT], F32, tag=pfx + "_ef", name=pfx + "_ef")
                nc.vector.tensor_copy(out=ef[:], in_=ei[:])
                mi = stats.tile([P, NT], I32, tag=pfx + "_mi", name=pfx + "_mi")
                nc.vector.tensor_scalar(
                    out=mi[:], in0=src[:].bitcast(I32),
                    scalar1=0x007FFFFF, scalar2=0x3F800000,
                    op0=Alu.bitwise_and, op1=Alu.bitwise_or,
                )
                nc.scalar.activation(out=dst[:], in_=mi[:].bitcast(F32), func=Act.Ln)
                ef2 = stats.tile([P, NT], F32, tag=pfx + "_ef2", name=pfx + "_ef2")
                nc.vector.tensor_scalar(
                    out=ef2[:], in0=ef[:], scalar1=LN2, scalar2=127.0 * LN2,
                    op0=Alu.mult, op1=Alu.subtract,
                )
                nc.vector.tensor_add(out=dst[:], in0=dst[:], in1=ef2[:])

            ln_ses = ftile("ln_ses")
            full_ln(ln_ses, se_s_all, "ls")
            ln_set = ftile("ln_set")
            full_ln(ln_set, se_t_all, "lt")
            kl = ftile("kl")
            nc.vector.tensor_mul(out=kl[:], in0=dot_all[:], in1=recip_t[:])
            nc.vector.tensor_add(out=kl[:], in0=kl[:], in1=nm_t_all[:])
            nc.vector.tensor_sub(out=kl[:], in0=kl[:], in1=nm_s_all[:])
            nc.vector.tensor_add(out=kl[:], in0=kl[:], in1=ln_ses[:])
            nc.vector.tensor_sub(out=kl[:], in0=kl[:], in1=ln_set[:])
            nc.vector.tensor_mul(out=kl[:], in0=kl[:], in1=maskT[:])

            packed = stats.tile([P, 4], F32, tag="packed", name="packed")
            fm = ftile("fm")
            nc.vector.tensor_mul(out=fm[:], in0=rsq_pos_all[:], in1=maskT[:])
            nc.vector.reduce_sum(out=packed[:, 0:1], in_=fm[:], axis=AxX)

            posd = ftile("posd")
            nc.scalar.activation(out=posd[:], in_=rsq_pos_all[:], func=Act.Sqrt)
            negd = ftile("negd")
            nc.scalar.activation(out=negd[:], in_=rsq_neg_all[:], func=Act.Sqrt)
            trip = ftile("trip")
            nc.vector.tensor_sub(out=trip[:], in0=posd[:], in1=negd[:])
            nc.scalar.activation(out=trip[:], in_=trip[:], func=Act.Relu, bias=b_margin[:])
            nc.vector.tensor_mul(out=trip[:], in0=trip[:], in1=maskT[:])
            nc.vector.reduce_sum(out=packed[:, 1:2], in_=trip[:], axis=AxX)

            nc.vector.reduce_sum(out=packed[:, 2:3], in_=kl[:], axis=AxX)

            vq = ftile("vq")
            nc.vector.tensor_add(out=vq[:], in0=x2_all[:], in1=nm_s_all[:])
            nc.vector.reduce_sum(out=packed[:, 3:4], in_=vq[:], axis=AxX)

            # partition reduce via fp32 matmul with ones
            pfin = psum.tile([1, 4], F32, tag="ps", name="pfin")
            nc.tensor.matmul(out=pfin[:], lhsT=ones[:], rhs=packed[:], start=True, stop=True)
            out_sb = stats.tile([1, 4], F32, tag="out_sb", name="out_sb")
            nc.scalar.copy(out=out_sb[:], in_=pfin[:])
            nc.sync.dma_start(out=out_d[:], in_=out_sb[:])

            if dump:
                arrs = [nm_t_all, nm_s_all, se_t_all, se_s_all, dot_all,
                        rsq_pos_all, rsq_neg_all, x2_all,
                        recip_t, ln_ses, ln_set, kl]
                for ai, arr in enumerate(arrs):
                    nc.sync.dma_start(
                        out=dbg_d[:, ai * NT : (ai + 1) * NT], in_=arr[:]
                    )

    _split_sync_waits(nc)
    return nc


_NC_CACHE = {}


def _get_nc(dump=False):
    key = "dump" if dump else "nc"
    if key not in _NC_CACHE:
        _NC_CACHE[key] = _build(dump=dump)
    return _NC_CACHE[key]


def _pack_w(X2):
    """(T, D) fp32 pre-scaled -> [128, NT*4*125] fp16 chunks per (it, d)."""
    # arr[it, p, d, row] -> [row? ] ; want w[row, it, d, p]
    a = X2.reshape(NT, P, 4, 128).transpose(3, 0, 2, 1)  # (128, NT, 4, P)
    return np.ascontiguousarray(a.astype(np.float16))


def kernel(student_features, teacher_features, teacher_codes, codebook, lengths,
           _debug=False, _trace=False, _dump=False):
    S = np.ascontiguousarray(np.asarray(student_features, dtype=np.float32))
    Tt = np.ascontiguousarray(np.asarray(teacher_features, dtype=np.float32))
    C = np.ascontiguousarray(np.asarray(codebook, dtype=np.float32))
    lengths = np.asarray(lengths)

    valid = np.minimum(lengths.astype(np.int64) // STRIDE, T)
    mask = (np.arange(T)[None, :] < valid[:, None]).astype(np.float32)  # (B,T)
    msum = float(mask.sum(dtype=np.float64))

    # packed codebook [128, 4*K] fp16 and c2 row
    ctpack = np.ascontiguousarray(
        C.reshape(K, 4, 128).transpose(2, 1, 0).reshape(128, 4 * K)
    ).astype(np.float16)
    c2 = (C.astype(np.float64) ** 2).sum(1).astype(np.float32)
    c2b = np.ascontiguousarray(np.broadcast_to(c2[None, :], (128, K)))

    in_maps = []
    for b in range(B):
        wt = _pack_w(2.0 * Tt[b])   # f=0: teacher
        ws = _pack_w(2.0 * S[b])    # f=1: student
        # interleave per tile: [row, it, f, d, p] -> [row, NT*1000]
        wpack = np.stack([wt, ws], axis=2)  # (128, NT, 2, 4, P)
        wpack = np.ascontiguousarray(wpack.reshape(128, NT * 1000))
        in_maps.append(
            {
                "s_nat": np.ascontiguousarray(S[b]),
                "t_nat": np.ascontiguousarray(Tt[b]),
                "tp_nat": np.ascontiguousarray(Tt[(b - 1) % B]),
                "wpack": wpack,
                "ctpack": ctpack,
                "c2b": c2b,
                "maskT": np.ascontiguousarray(mask[b].reshape(NT, P).T),
            }
        )

    nc = _get_nc(dump=_dump)
    res = run_bass_kernel_spmd(nc, in_maps, core_ids=list(range(NC)), trace=_trace)
    parts = np.stack([res.results[b]["partials"][0] for b in range(B)])  # (B,4)
    if _dump:
        dbg = np.stack([res.results[b]["dbg"] for b in range(B)])
        return parts, dbg

    F_sum, TR_sum, KL_sum, Q_sum = parts.astype(np.float64).sum(0)
    total = (
        F_sum / D / msum
        + TR_sum / msum
        + KL_sum / msum
        + 0.2 * Q_sum / (B * T * D)
    )
    out = np.array(total, dtype=np.float32)
    if _debug and _trace:
        return out, parts, res.exec_time_ns
    if _debug:
        return out, parts
    return out
